# revision 48
# baseline (speedup 1.0000x reference)
"""Trainium2 Bass kernel for nn_PositionalEmbedding (embedding-lookup form).

Math: out[b, 2j]   = mean_k sin(params[k] * dc[b,k] * inv_freq[j])
      out[b, 2j+1] = mean_k cos(params[k] * dc[b,k] * inv_freq[j])

dc[b,k] are integers in [0, 60), so sin/cos over all (k, value) pairs form a
360-row lookup table (pre-scaled 1/6, bf16) computed on the HOST from
`params`.  The batch reduction becomes, per 128-row tile,
out_tile = onehotT.T @ T accumulated over 3 K-chunks of 120 dictionary rows.

The PE sequencer issues ~4.6M instr/s (one matmul per ~216ns regardless of
size), so instruction COUNT per 512-row group is the scarce resource.  The
dictionary is laid out so chunk c, partition p holds (component p%6, value
20c + p//6): the replicated components crep[p,b] = dc[b, p%6] are then the
SAME for all 3 chunks -> ONE replication matmul per group (instead of 3)
feeding 3 is_equal ops against per-chunk value columns.  13 PE instructions
per group (~2.8us) ~= the 2.8us/group DMA floor for the fp32 output.

Per group of 512 output rows (4 PSUM tiles): PSUM->SBUF copies go to ACT
(GPSIMD cannot access PSUM on TRN2), the one-hot pipeline runs two groups
ahead so DVE's 3 serial is_equal overlap main matmuls, and the 4 tiles
leave through ONE dma_start (SP's DGE config costs 565ns per dma_start).
Batch rows are pre-permuted on the host so DRAM row (4p + h) of a group
maps to stationary column p of tile-slot h: each DMA descriptor then
covers 4 consecutive DRAM rows (8KB contiguous) from one SBUF partition.

dct is uploaded as [128, 4096] (8KB/partition; a [6, 16384] layout would
bottleneck on the ~5.4 B/ns per-partition SBUF write port for ~6us), with
component rows at base partitions {0,32,64,96} because matmul tile
positions must be multiples of 32.

Data parallel over 8 NeuronCores: each core handles 16384 rows.
"""

import numpy as np
import ml_dtypes

B = 131072
D = 512
NCOMP = 6
HYPER = 2100.0
NCORES = 8
BL = B // NCORES          # 16384 rows per core
P = 128                   # partitions / rows per output tile
NV = 60                   # dictionary values per component
ND = NCOMP * NV           # 360 dictionary rows
CK = 120                  # dictionary rows per K-chunk
NCHUNK = ND // CK         # 3
NVC = NV // NCHUNK        # 20 values per component per chunk
GROUP = 4                 # output tiles per one-hot group (512 batch cols)
# dct partition-blocks: matmul operands may only start at base partition
# 0/32/64 (bass_rust lowering limit).  Six blocks of ~5 groups live at
# (base 32*(b%3), sub-rows 6*(b//3)) — the K=12 replication stationary
# zeroes the other sub-block's rows — bounding the per-partition SBUF
# write-port time (~5.4 B/ns) for the dct upload to ~1.1us.
DCTB = 6
GS2 = (0, 6, 12, 18, 24, 28, 32)  # group ranges per block
NWARM = 20                # PE p-state warmup matmuls (2.4GHz needs ~3us busy)

_CACHE: dict = {}


def _build_nc(bl):
    import concourse.bacc as bacc
    import concourse.mybir as mybir
    from concourse import tile

    f32 = mybir.dt.float32
    f16 = mybir.dt.bfloat16
    Alu = mybir.AluOpType

    ntiles = bl // P
    ngroups = ntiles // GROUP             # 32
    colb = (GS2[1] - GS2[0]) * GROUP * P  # dct cols in widest block (3072)

    nc = bacc.Bacc(trn_type="TRN2")
    dct = nc.dram_tensor("dct", [DCTB * NCOMP, colb], f16, kind="ExternalInput").ap()
    r12d = nc.dram_tensor("r12d", [P, DCTB * CK], f16,
                          kind="ExternalInput").ap()
    vvd = nc.dram_tensor("vvd", [CK, NCHUNK], f32, kind="ExternalInput").ap()
    tbd = nc.dram_tensor("tbd", [CK, NCHUNK * D], f16, kind="ExternalInput").ap()
    out = nc.dram_tensor("out", [bl, D], f32, kind="ExternalOutput").ap()

    with tile.TileContext(nc) as tc:
        with (
            tc.tile_pool(name="const", bufs=1) as cpool,
            tc.tile_pool(name="oh", bufs=4) as ohpool,
            tc.tile_pool(name="osb", bufs=4) as opool,
            # ONE shared 8-bank PSUM pool for both the replication output and
            # the 4 output tiles (5 allocations/group): the rotation spreads
            # every PSUM WAR 1.6 groups back.  Dedicated crep/ps pools (2+6
            # or 3+5) always leave one edge with zero slack (~0.4us/group
            # PE stall on either the is_equal or the copy semaphore).
            tc.tile_pool(name="mm", bufs=8, space="PSUM") as qpool,
        ):
            # dct lands as 3 dense 12-row blocks into base partitions
            # 0/32/64 (each holding two 6-row sub-blocks).  The tile is a
            # full 128 partitions because the replication matmul contracts
            # K=128 (zero stationary rows outside the component rows):
            # K=12 at base 32b would need tile_size (32,128)@(32b,0), and
            # switching the PE array tile config between the replication
            # and the (128,128)@(0,0) main matmuls costs a ~0.65us
            # pipeline flush per group.  The unused partitions are zeroed
            # once so the K=128 read is initialized.
            # NOTE: partitions the block DMAs don't write are read by the
            # K=128 repl matmul with zero stationary weights — their values
            # never reach the output (left uninitialized on purpose; a
            # memset would cost ~2us of DVE before the dct DMAs can land).
            dct_sb = cpool.tile([P, colb], f16, tag="dct")
            for jb in range(3):
                nc.sync.dma_start(
                    out=dct_sb[32 * jb:32 * jb + 2 * NCOMP, :],
                    in_=dct[2 * NCOMP * jb:2 * NCOMP * (jb + 1), :],
                )
                if jb == 0:
                    r12_sb = cpool.tile([P, DCTB * CK], f16, tag="r12")
                    nc.sync.dma_start(out=r12_sb[:, :], in_=r12d)
                    vv_sb = cpool.tile([CK, NCHUNK], f32, tag="vv")
                    nc.sync.dma_start(out=vv_sb[:, :], in_=vvd)
            tb_sb = cpool.tile([CK, NCHUNK * D], f16, tag="tb")
            nc.scalar.dma_start(out=tb_sb[:, :], in_=tbd)

            def emit_onehot(g):
                b = max(i for i in range(DCTB) if GS2[i] <= g)
                c0 = (g - GS2[b]) * GROUP * P
                crep_t = qpool.tile([P, GROUP * P], f32, tag="mm")
                crep = crep_t[0:CK, :]
                nc.tensor.matmul(
                    crep,
                    r12_sb[:, b * CK:(b + 1) * CK],
                    dct_sb[:, c0:c0 + GROUP * P],
                    start=True, stop=True,
                )
                oh = ohpool.tile([CK, NCHUNK * GROUP * P], f16, tag="oh")
                for c in range(NCHUNK):
                    nc.vector.tensor_scalar(
                        out=oh[:, c * GROUP * P:(c + 1) * GROUP * P],
                        in0=crep,
                        scalar1=vv_sb[:, c:c + 1], scalar2=None,
                        op0=Alu.is_equal,
                    )
                return oh

            oh_q = [emit_onehot(0), emit_onehot(1), emit_onehot(2)]
            for g in range(ngroups):
                oh = oh_q.pop(0)
                ob = opool.tile([P, GROUP * D], f32, tag="ob")
                pss = []
                for t in range(GROUP):
                    ps = qpool.tile([P, D], f32, tag="mm")
                    for c in range(NCHUNK):
                        nc.tensor.matmul(
                            ps[:, :],
                            oh[:, c * GROUP * P + t * P:c * GROUP * P + (t + 1) * P],
                            tb_sb[:, c * D:(c + 1) * D],
                            start=(c == 0), stop=(c == NCHUNK - 1),
                        )
                    pss.append(ps)
                if g + 3 < ngroups:
                    oh_q.append(emit_onehot(g + 3))
                if g == 0:
                    # per-tile DMAs compress pipeline fill (~2us); SP has
                    # idle config capacity here.  Tile-slot t holds DRAM
                    # rows g*512 + 4p + t.
                    for t in range(GROUP):
                        nc.scalar.copy(ob[:, t * D:(t + 1) * D], pss[t][:, :])
                        nc.sync.dma_start(
                            out=out[t:GROUP * P:GROUP, :],
                            in_=ob[:, t * D:(t + 1) * D],
                        )
                else:
                    for t in range(GROUP):
                        nc.scalar.copy(ob[:, t * D:(t + 1) * D], pss[t][:, :])
                    nc.sync.dma_start(
                        out=out[g * GROUP * P:(g + 1) * GROUP * P, :].rearrange(
                            "(p h) d -> p (h d)", h=GROUP),
                        in_=ob[:, :],
                    )

    # Bacc legalization: splits multi-sync-waits into EventSemaphores
    # (walrus allows at most one wait per instruction), allocates registers.
    nc.compile()
    return nc


def _get_nc(bl=BL):
    key = ("nc", bl)
    if key not in _CACHE:
        _CACHE[key] = _build_nc(bl)
    return _CACHE[key]


def _host_constants(prm):
    """Lookup table (pre-scaled 1/6), replication matrices, value columns.

    Dictionary layout: chunk c, partition p <-> (component p%6, value
    20c + p//6).
    """
    j = np.arange(0, D, 2, dtype=np.float32)
    inv_freq = np.float32(HYPER) ** (
        -(np.float32(2.0) * (j + np.float32(1.0))) / np.float32(D))  # [256] f32
    p_idx = np.arange(CK)
    kk = p_idx % NCOMP                     # component per partition
    inv6 = np.float32(1.0 / NCOMP)
    tb = np.empty((CK, NCHUNK * D), np.float32)
    vv = np.empty((CK, NCHUNK), np.float32)
    for c in range(NCHUNK):
        vals = (NVC * c + p_idx // NCOMP).astype(np.float32)
        vv[:, c] = vals
        pv = prm[kk] * vals                                   # [120] f32
        phase = pv[:, None] * inv_freq[None, :]               # [120, 256] f32
        tb[:, c * D + 0:c * D + D:2] = np.sin(phase) * inv6
        tb[:, c * D + 1:c * D + D:2] = np.cos(phase) * inv6
    tb16 = tb.astype(ml_dtypes.bfloat16)

    # K=128 replication stationaries, one 120-col variant per dct block b:
    # rows 32*(b%3) + 6*(b//3) + k carry (p%6==k), everything else zero.
    r12 = np.zeros((P, DCTB * CK), np.float32)
    for b in range(DCTB):
        bb, sub = b % 3, b // 3
        for k in range(NCOMP):
            r12[32 * bb + NCOMP * sub + k, b * CK + np.where(kk == k)[0]] = 1.0
    r12 = r12.astype(ml_dtypes.bfloat16)
    return tb16, r12, vv


def _in_maps(date_components, params):
    dc = np.asarray(date_components).astype(np.int32, copy=False)
    prm = np.asarray(params).astype(np.float32, copy=False).reshape(NCOMP)
    tb16, r12, vv = _host_constants(prm)

    # batch permutation: stationary column p of tile-slot h in group g holds
    # original row g*512 + 4p + h, so the group's single out-DMA writes DRAM
    # rows in natural order with 4-row-contiguous descriptors.
    jj = np.arange(GROUP * P)
    src = GROUP * (jj % P) + (jj // P)
    perm = (np.arange(0, BL, GROUP * P)[:, None] + src[None, :]).ravel()

    colb = (GS2[1] - GS2[0]) * GROUP * P
    maps = []
    for i in range(NCORES):
        shard = dc[i * BL:(i + 1) * BL]
        dctt = np.ascontiguousarray(shard[perm].T)            # [6, BL]
        # dense [36, 3072]: device DMA jb places dense rows 12jb..12jb+11 at
        # base partition 32jb; those 12 rows = sub-blocks (bb=jb, sub=0|1)
        # = group blocks b=jb and b=jb+3.
        dctm = np.zeros((DCTB * NCOMP, colb), np.float32)
        for b in range(DCTB):
            bb, sub = b % 3, b // 3
            lo = GS2[b] * GROUP * P
            hi = GS2[b + 1] * GROUP * P
            r0 = 2 * NCOMP * bb + NCOMP * sub
            dctm[r0:r0 + NCOMP, 0:hi - lo] = dctt[:, lo:hi]
        maps.append({
            "dct": dctm.astype(ml_dtypes.bfloat16),
            "r12d": r12,
            "vvd": vv,
            "tbd": tb16,
        })
    return maps


def kernel(date_components, params, _trace=False):
    from concourse.bass_utils import run_bass_kernel_spmd

    nc = _get_nc()
    maps = _in_maps(date_components, params)
    res = run_bass_kernel_spmd(
        nc, maps, core_ids=list(range(NCORES)),
        trace=_trace, trace_cores=[0] if _trace else None,
    )
    kernel.last_results = res
    return np.concatenate([r["out"] for r in res.results], axis=0)


# revision 49
# speedup vs baseline: 1.0055x; 1.0055x over previous
"""Trainium2 Bass kernel for nn_PositionalEmbedding (embedding-lookup form).

Math: out[b, 2j]   = mean_k sin(params[k] * dc[b,k] * inv_freq[j])
      out[b, 2j+1] = mean_k cos(params[k] * dc[b,k] * inv_freq[j])

dc[b,k] are integers in [0, 60), so sin/cos over all (k, value) pairs form a
360-row lookup table (pre-scaled 1/6, bf16) computed on the HOST from
`params`.  The batch reduction becomes, per 128-row tile,
out_tile = onehotT.T @ T accumulated over 3 K-chunks of 120 dictionary rows.

The PE sequencer issues ~4.6M instr/s (one matmul per ~216ns regardless of
size), so instruction COUNT per 512-row group is the scarce resource.  The
dictionary is laid out so chunk c, partition p holds (component p%6, value
20c + p//6): the replicated components crep[p,b] = dc[b, p%6] are then the
SAME for all 3 chunks -> ONE replication matmul per group (instead of 3)
feeding 3 is_equal ops against per-chunk value columns.  13 PE instructions
per group (~2.8us) ~= the 2.8us/group DMA floor for the fp32 output.

Per group of 512 output rows (4 PSUM tiles): PSUM->SBUF copies go to ACT
(GPSIMD cannot access PSUM on TRN2), the one-hot pipeline runs two groups
ahead so DVE's 3 serial is_equal overlap main matmuls, and the 4 tiles
leave through ONE dma_start (SP's DGE config costs 565ns per dma_start).
Batch rows are pre-permuted on the host so DRAM row (4p + h) of a group
maps to stationary column p of tile-slot h: each DMA descriptor then
covers 4 consecutive DRAM rows (8KB contiguous) from one SBUF partition.

dct is uploaded as [128, 4096] (8KB/partition; a [6, 16384] layout would
bottleneck on the ~5.4 B/ns per-partition SBUF write port for ~6us), with
component rows at base partitions {0,32,64,96} because matmul tile
positions must be multiples of 32.

Data parallel over 8 NeuronCores: each core handles 16384 rows.
"""

import numpy as np
import ml_dtypes

B = 131072
D = 512
NCOMP = 6
HYPER = 2100.0
NCORES = 8
BL = B // NCORES          # 16384 rows per core
P = 128                   # partitions / rows per output tile
NV = 60                   # dictionary values per component
ND = NCOMP * NV           # 360 dictionary rows
CK = 120                  # dictionary rows per K-chunk
NCHUNK = ND // CK         # 3
NVC = NV // NCHUNK        # 20 values per component per chunk
GROUP = 4                 # output tiles per one-hot group (512 batch cols)
# dct partition-blocks: matmul operands may only start at base partition
# 0/32/64 (bass_rust lowering limit).  Six blocks of ~5 groups live at
# (base 32*(b%3), sub-rows 6*(b//3)) — the K=12 replication stationary
# zeroes the other sub-block's rows — bounding the per-partition SBUF
# write-port time (~5.4 B/ns) for the dct upload to ~1.1us.
DCTB = 6
GS2 = (0, 6, 12, 18, 24, 28, 32)  # group ranges per block
NWARM = 20                # PE p-state warmup matmuls (2.4GHz needs ~3us busy)

_CACHE: dict = {}


def _build_nc(bl):
    import concourse.bacc as bacc
    import concourse.mybir as mybir
    from concourse import tile

    f32 = mybir.dt.float32
    f16 = mybir.dt.bfloat16
    Alu = mybir.AluOpType

    ntiles = bl // P
    ngroups = ntiles // GROUP             # 32
    colb = (GS2[1] - GS2[0]) * GROUP * P  # dct cols in widest block (3072)

    nc = bacc.Bacc(trn_type="TRN2")
    dct = nc.dram_tensor("dct", [DCTB * NCOMP, colb], f16, kind="ExternalInput").ap()
    r12d = nc.dram_tensor("r12d", [P, DCTB * CK], f16,
                          kind="ExternalInput").ap()
    vvd = nc.dram_tensor("vvd", [CK, NCHUNK], f32, kind="ExternalInput").ap()
    tbd = nc.dram_tensor("tbd", [CK, NCHUNK * D], f16, kind="ExternalInput").ap()
    out = nc.dram_tensor("out", [bl, D], f32, kind="ExternalOutput").ap()

    with tile.TileContext(nc) as tc:
        with (
            tc.tile_pool(name="const", bufs=1) as cpool,
            tc.tile_pool(name="oh", bufs=4) as ohpool,
            tc.tile_pool(name="osb", bufs=4) as opool,
            # ONE shared 8-bank PSUM pool for both the replication output and
            # the 4 output tiles (5 allocations/group): the rotation spreads
            # every PSUM WAR 1.6 groups back.  Dedicated crep/ps pools (2+6
            # or 3+5) always leave one edge with zero slack (~0.4us/group
            # PE stall on either the is_equal or the copy semaphore).
            tc.tile_pool(name="mm", bufs=8, space="PSUM") as qpool,
        ):
            # dct lands as 3 dense 12-row blocks into base partitions
            # 0/32/64 (each holding two 6-row sub-blocks).  The tile is a
            # full 128 partitions because the replication matmul contracts
            # K=128 (zero stationary rows outside the component rows):
            # K=12 at base 32b would need tile_size (32,128)@(32b,0), and
            # switching the PE array tile config between the replication
            # and the (128,128)@(0,0) main matmuls costs a ~0.65us
            # pipeline flush per group.  The unused partitions are zeroed
            # once so the K=128 read is initialized.
            dct_sb = cpool.tile([P, colb], f16, tag="dct")
            nc.vector.memset(dct_sb[:, :], 0.0)
            for jb in range(3):
                nc.sync.dma_start(
                    out=dct_sb[32 * jb:32 * jb + 2 * NCOMP, :],
                    in_=dct[2 * NCOMP * jb:2 * NCOMP * (jb + 1), :],
                )
                if jb == 0:
                    r12_sb = cpool.tile([P, DCTB * CK], f16, tag="r12")
                    nc.sync.dma_start(out=r12_sb[:, :], in_=r12d)
                    vv_sb = cpool.tile([CK, NCHUNK], f32, tag="vv")
                    nc.sync.dma_start(out=vv_sb[:, :], in_=vvd)
            tb_sb = cpool.tile([CK, NCHUNK * D], f16, tag="tb")
            nc.scalar.dma_start(out=tb_sb[:, :], in_=tbd)

            def emit_onehot(g):
                b = max(i for i in range(DCTB) if GS2[i] <= g)
                c0 = (g - GS2[b]) * GROUP * P
                crep_t = qpool.tile([P, GROUP * P], f32, tag="mm")
                crep = crep_t[0:CK, :]
                nc.tensor.matmul(
                    crep,
                    r12_sb[:, b * CK:(b + 1) * CK],
                    dct_sb[:, c0:c0 + GROUP * P],
                    start=True, stop=True,
                )
                oh = ohpool.tile([CK, NCHUNK * GROUP * P], f16, tag="oh")
                for c in range(NCHUNK):
                    nc.vector.tensor_scalar(
                        out=oh[:, c * GROUP * P:(c + 1) * GROUP * P],
                        in0=crep,
                        scalar1=vv_sb[:, c:c + 1], scalar2=None,
                        op0=Alu.is_equal,
                    )
                return oh

            oh_q = [emit_onehot(0), emit_onehot(1), emit_onehot(2)]
            for g in range(ngroups):
                oh = oh_q.pop(0)
                ob = opool.tile([P, GROUP * D], f32, tag="ob")
                pss = []
                for t in range(GROUP):
                    ps = qpool.tile([P, D], f32, tag="mm")
                    for c in range(NCHUNK):
                        nc.tensor.matmul(
                            ps[:, :],
                            oh[:, c * GROUP * P + t * P:c * GROUP * P + (t + 1) * P],
                            tb_sb[:, c * D:(c + 1) * D],
                            start=(c == 0), stop=(c == NCHUNK - 1),
                        )
                    pss.append(ps)
                if g + 3 < ngroups:
                    oh_q.append(emit_onehot(g + 3))
                if g == 0:
                    # per-tile DMAs compress pipeline fill (~2us); SP has
                    # idle config capacity here.  Tile-slot t holds DRAM
                    # rows g*512 + 4p + t.
                    for t in range(GROUP):
                        nc.scalar.copy(ob[:, t * D:(t + 1) * D], pss[t][:, :])
                        nc.sync.dma_start(
                            out=out[t:GROUP * P:GROUP, :],
                            in_=ob[:, t * D:(t + 1) * D],
                        )
                else:
                    for t in range(GROUP):
                        nc.scalar.copy(ob[:, t * D:(t + 1) * D], pss[t][:, :])
                    nc.sync.dma_start(
                        out=out[g * GROUP * P:(g + 1) * GROUP * P, :].rearrange(
                            "(p h) d -> p (h d)", h=GROUP),
                        in_=ob[:, :],
                    )

    # Bacc legalization: splits multi-sync-waits into EventSemaphores
    # (walrus allows at most one wait per instruction), allocates registers.
    nc.compile()
    return nc


def _get_nc(bl=BL):
    key = ("nc", bl)
    if key not in _CACHE:
        _CACHE[key] = _build_nc(bl)
    return _CACHE[key]


def _host_constants(prm):
    """Lookup table (pre-scaled 1/6), replication matrices, value columns.

    Dictionary layout: chunk c, partition p <-> (component p%6, value
    20c + p//6).
    """
    j = np.arange(0, D, 2, dtype=np.float32)
    inv_freq = np.float32(HYPER) ** (
        -(np.float32(2.0) * (j + np.float32(1.0))) / np.float32(D))  # [256] f32
    p_idx = np.arange(CK)
    kk = p_idx % NCOMP                     # component per partition
    inv6 = np.float32(1.0 / NCOMP)
    tb = np.empty((CK, NCHUNK * D), np.float32)
    vv = np.empty((CK, NCHUNK), np.float32)
    for c in range(NCHUNK):
        vals = (NVC * c + p_idx // NCOMP).astype(np.float32)
        vv[:, c] = vals
        pv = prm[kk] * vals                                   # [120] f32
        phase = pv[:, None] * inv_freq[None, :]               # [120, 256] f32
        tb[:, c * D + 0:c * D + D:2] = np.sin(phase) * inv6
        tb[:, c * D + 1:c * D + D:2] = np.cos(phase) * inv6
    tb16 = tb.astype(ml_dtypes.bfloat16)

    # K=128 replication stationaries, one 120-col variant per dct block b:
    # rows 32*(b%3) + 6*(b//3) + k carry (p%6==k), everything else zero.
    r12 = np.zeros((P, DCTB * CK), np.float32)
    for b in range(DCTB):
        bb, sub = b % 3, b // 3
        for k in range(NCOMP):
            r12[32 * bb + NCOMP * sub + k, b * CK + np.where(kk == k)[0]] = 1.0
    r12 = r12.astype(ml_dtypes.bfloat16)
    return tb16, r12, vv


def _in_maps(date_components, params):
    dc = np.asarray(date_components).astype(np.int32, copy=False)
    prm = np.asarray(params).astype(np.float32, copy=False).reshape(NCOMP)
    tb16, r12, vv = _host_constants(prm)

    # batch permutation: stationary column p of tile-slot h in group g holds
    # original row g*512 + 4p + h, so the group's single out-DMA writes DRAM
    # rows in natural order with 4-row-contiguous descriptors.
    jj = np.arange(GROUP * P)
    src = GROUP * (jj % P) + (jj // P)
    perm = (np.arange(0, BL, GROUP * P)[:, None] + src[None, :]).ravel()

    colb = (GS2[1] - GS2[0]) * GROUP * P
    maps = []
    for i in range(NCORES):
        shard = dc[i * BL:(i + 1) * BL]
        dctt = np.ascontiguousarray(shard[perm].T)            # [6, BL]
        # dense [36, 3072]: device DMA jb places dense rows 12jb..12jb+11 at
        # base partition 32jb; those 12 rows = sub-blocks (bb=jb, sub=0|1)
        # = group blocks b=jb and b=jb+3.
        dctm = np.zeros((DCTB * NCOMP, colb), np.float32)
        for b in range(DCTB):
            bb, sub = b % 3, b // 3
            lo = GS2[b] * GROUP * P
            hi = GS2[b + 1] * GROUP * P
            r0 = 2 * NCOMP * bb + NCOMP * sub
            dctm[r0:r0 + NCOMP, 0:hi - lo] = dctt[:, lo:hi]
        maps.append({
            "dct": dctm.astype(ml_dtypes.bfloat16),
            "r12d": r12,
            "vvd": vv,
            "tbd": tb16,
        })
    return maps


def kernel(date_components, params, _trace=False):
    from concourse.bass_utils import run_bass_kernel_spmd

    nc = _get_nc()
    maps = _in_maps(date_components, params)
    res = run_bass_kernel_spmd(
        nc, maps, core_ids=list(range(NCORES)),
        trace=_trace, trace_cores=[0] if _trace else None,
    )
    kernel.last_results = res
    return np.concatenate([r["out"] for r in res.results], axis=0)
